# revision 1
# baseline (speedup 1.0000x reference)
"""Trainium2 Bass kernel for nn_MultiHeadGlobalAttention.

Math (B=64, N=4096, C=128, H=4):
  mask[b,n] = n < graph_size[b]
  Vg = (V @ weight + bias).reshape(B,N,H,C)
  a[b,n,h] = sum_c Vg[b,n,h,c] * tune[0,h,c]   -> leaky_relu -> masked softmax over n
  out[b] = (sum_n a[b,n,h] * Vg[b,n,h,:]).reshape(H*C)

Key reduction: softmax weights sum to 1, so
  out[b, h*C:(h+1)*C] = (sum_n e[n,h] * V[b,n,:]) / Z[b,h] @ W[:, h*C:(h+1)*C] + bias[h*C:(h+1)*C]
with logits l[n,h] = V[b,n,:] @ w2[:,h] + b2[h], w2 = sum_d W[:,h*C+d]*tune[h,d],
b2 = sum_d bias[h*C+d]*tune[h,d], e = max(exp(l), exp(ALPHA*l)) * mask01.
Vg is never materialized; V is read once per node.

Sharding: 64 graphs -> 8 cores x 8 slots. Graphs sorted by chunk count
(ceil(gs/128)) descending; rank group g -> slot g, one graph per core.
Slot cap = max chunks in group -> uniform SPMD program across cores.

Device schedule (per-instruction overheads dominate -> batch everything):
  - V shipped partition-major [128, nchunk*128]; loaded in GD-chunk DMAs.
  - logits accumulated into a [128, GE*4] PSUM region per GE-chunk group;
    2 batched Exps + 1 max + 1 mask-mul per group.
  - V^T via PE transpose, 4 chunks per PSUM bank, single copy-back
    alternating DVE/ACT.
  - e-weighted V accumulation matmuls emitted one group late so the
    in-order PE queue never stalls on the exp chain.
"""

import math
import numpy as np

import concourse.bass as bass
import concourse.tile as tile
from concourse import bacc, mybir
from concourse.bass_utils import run_bass_kernel_spmd
from concourse.masks import make_identity

B, N, C, H = 64, 4096, 128, 4
P = 128          # chunk size == partition count
ALPHA = 0.2      # leaky_relu negative slope
NCORES = 8
NSLOTS = B // NCORES
GD = 16          # chunks per V DMA
GE = 32          # chunks per exp/softmax group
F32 = mybir.dt.float32


def _plan(graph_size):
    """Sort graphs by chunk count ascending; rank group g -> slot g across
    cores. Ascending so the big slot is last in the stream: every other
    slot's tail work (Z, sall copy, out matmul, DMA) completes while V is
    still streaming, leaving only the big slot's short chain at the end."""
    nch = np.maximum(1, np.ceil(np.asarray(graph_size, np.int64) / P).astype(np.int64))
    order = np.argsort(nch, kind="stable")
    caps = [int(nch[order[g * NCORES + NCORES - 1]]) for g in range(NSLOTS)]
    offs = np.concatenate([[0], np.cumsum(caps)]).astype(np.int64)
    return order, caps, offs, int(offs[-1])


def _build_program(caps, nchunk):
    nc = bacc.Bacc("TRN2", target_bir_lowering=False, debug=False)

    vh_d = nc.declare_dram_parameter("Vh", [P, nchunk * P], F32, isOutput=False)
    em_d = nc.declare_dram_parameter("em", [P, nchunk * H], F32, isOutput=False)
    wb_d = nc.declare_dram_parameter("wb", [C, 2 * H], F32, isOutput=False)
    wm_d = nc.declare_dram_parameter("wmat", [C, H * C], F32, isOutput=False)
    raw_d = nc.declare_dram_parameter("raw", [NSLOTS * H, H * C], F32, isOutput=True)
    zc_d = nc.declare_dram_parameter("zc", [P, NSLOTS], F32, isOutput=True)

    offs = [0]
    for cp in caps:
        offs.append(offs[-1] + cp)
    slot_of = np.zeros(nchunk, np.int64)
    for g in range(NSLOTS):
        slot_of[offs[g]:offs[g + 1]] = g
    # V DMA group sizes: small ramp-up so the first transposes start early,
    # big middle groups to amortize per-DMA overhead, halving ramp-down so
    # the final chunks arrive incrementally and the tail chain is short.
    sizes = []
    rem = nchunk
    for s in (4, 4, 8):
        if rem <= 0:
            break
        t = min(s, rem)
        sizes.append(t)
        rem -= t
    while rem > 48:
        sizes.append(32)
        rem -= 32
    while rem > 2:
        t = max(2, rem // 2)
        sizes.append(t)
        rem -= t
    if rem:
        sizes.append(rem)
    gstart = [0]
    for s in sizes:
        gstart.append(gstart[-1] + s)
    loc = []
    for k, s in enumerate(sizes):
        for r in range(s):
            loc.append((k, r))
    ngd = len(sizes)
    # softmax group boundaries: 32-wide while plenty remains, halving tail
    geb = [0]
    rem = nchunk
    while rem > 64:
        geb.append(geb[-1] + 32)
        rem -= 32
    while rem > 4:
        t = max(4, rem // 2)
        geb.append(geb[-1] + t)
        rem -= t
    if rem:
        geb.append(geb[-1] + rem)
    nge = len(geb) - 1

    with tile.TileContext(nc) as tc:
        with (
            tc.tile_pool(name="consts", bufs=1) as consts,
            tc.tile_pool(name="vres", bufs=ngd) as vres,
            tc.tile_pool(name="vt4", bufs=3) as vt4p,
            tc.tile_pool(name="e12", bufs=6) as e12p,
            tc.tile_pool(name="eallp", bufs=1) as eall_pool,
            tc.tile_pool(name="outp", bufs=1) as out_pool,
            tc.tile_pool(name="ps_vt", bufs=2, space="PSUM") as ps_vt,
            tc.tile_pool(name="ps_l", bufs=2, space="PSUM") as ps_l,
            tc.tile_pool(name="ps_s", bufs=1, space="PSUM") as ps_s,
            tc.tile_pool(name="ps_o", bufs=2, space="PSUM") as ps_o,
            tc.tile_pool(name="ps_z", bufs=1, space="PSUM") as ps_z,
        ):
            # DMA order matters: transfers share one exclusive device, so
            # order = criticality. wb (w2|b2) gates the first logits; the
            # first (small) V groups gate the first transposes; em gates the
            # first softmax; wm is only needed once slots start completing.
            wb_sb = consts.tile([C, 2 * H], F32)
            nc.sync.dma_start(wb_sb[:], wb_d[:])
            w2_ap = wb_sb[:, 0:H]
            b2_ap = wb_sb[0:1, H:2 * H]

            vg = []
            for k in range(ngd):
                t = vres.tile([P, sizes[k] * P], F32)
                vg.append(t)
            nc.sync.dma_start(vg[0][:], vh_d[:, 0:sizes[0] * P])
            nc.sync.dma_start(
                vg[1][:], vh_d[:, gstart[1] * P:gstart[1] * P + sizes[1] * P]
            )

            em_sb = consts.tile([P, nchunk * H], F32)
            nc.sync.dma_start(em_sb[:], em_d[:])

            for k in range(2, ngd):
                a = gstart[k] * P
                nc.sync.dma_start(vg[k][:], vh_d[:, a:a + sizes[k] * P])

            wm_sb = consts.tile([C, H * C], F32)
            nc.sync.dma_start(wm_sb[:], wm_d[:])

            onesrow = consts.tile([1, P], F32)
            nc.gpsimd.memset(onesrow[:], 1.0)
            ident = consts.tile([P, P], F32)
            make_identity(nc, ident[:])
            ones = consts.tile([P, 1], F32)
            nc.gpsimd.memset(ones[:], 1.0)

            eall_sb = eall_pool.tile([P, nchunk * H], F32)
            psum_s = ps_s.tile([C, NSLOTS * H], F32)
            zc_ps = ps_z.tile([P, NSLOTS], F32)
            zc_sb = out_pool.tile([P, NSLOTS], F32)
            nc.gpsimd.memset(zc_sb[:], 0.0)
            sall = out_pool.tile([C, NSLOTS * H], F32)

            def vsl(j):
                k, r = loc[j]
                return vg[k][:, r * P:(r + 1) * P]

            rtog = [0]

            def emit_ready(c0, c1):
                # accumulation matmuls for chunks [c0, c1), then the full
                # output pipeline (Z partials, sall copy, out matmul, copy,
                # DMA) for any slot whose chunk span completes in this range
                for j in range(c0, c1):
                    g = int(slot_of[j])
                    nc.tensor.matmul(
                        psum_s[:, g * H:(g + 1) * H], vsl(j),
                        eall_sb[:, j * H:(j + 1) * H],
                        start=(j == offs[g]), stop=(j == offs[g + 1] - 1),
                    )
                for g in range(NSLOTS):
                    if not (c0 < offs[g + 1] <= c1):
                        continue
                    lo = offs[g] * H
                    hi = lo + caps[g] * H
                    nc.tensor.matmul(
                        zc_ps[0:caps[g] * H, g:g + 1], eall_sb[:, lo:hi],
                        ones[:], start=True, stop=True,
                    )
                    nc.vector.tensor_copy(
                        zc_sb[0:caps[g] * H, g:g + 1],
                        zc_ps[0:caps[g] * H, g:g + 1],
                    )
                    gl, gh = g * H, (g + 1) * H
                    nc.vector.tensor_copy(sall[:, gl:gh], psum_s[:, gl:gh])
                    if g < NSLOTS - 1:
                        rp = ps_o.tile([H, H * C], F32)
                        rb = out_pool.tile([H, H * C], F32)
                        nc.tensor.matmul(
                            rp[:], sall[:, gl:gh], wm_sb[:],
                            start=True, stop=True,
                        )
                        if rtog[0]:
                            nc.vector.tensor_copy(rb[:], rp[:])
                        else:
                            nc.scalar.activation(
                                rb[:], rp[:], mybir.ActivationFunctionType.Copy
                            )
                        rtog[0] ^= 1
                        nc.sync.dma_start(raw_d[gl:gh, :], rb[:])
                    else:
                        # last slot: per-head matmuls into one PSUM row so
                        # copies/DMA pipeline behind PE; host reads the
                        # diagonal blocks from raw row gl (see _assemble)
                        rp = ps_o.tile([1, H * C], F32)
                        rb = out_pool.tile([1, H * C], F32)
                        for h in range(H):
                            blk = slice(h * C, (h + 1) * C)
                            nc.tensor.matmul(
                                rp[0:1, blk], sall[:, gl + h:gl + h + 1],
                                wm_sb[:, blk], start=True, stop=True,
                            )
                            if h == 1:
                                nc.scalar.activation(
                                    rb[0:1, 0:2 * C], rp[0:1, 0:2 * C],
                                    mybir.ActivationFunctionType.Copy,
                                )
                            elif h == 3:
                                nc.vector.tensor_copy(
                                    rb[0:1, 2 * C:4 * C], rp[0:1, 2 * C:4 * C]
                                )
                        nc.sync.dma_start(zc_d[:], zc_sb[:])
                        nc.sync.dma_start(raw_d[gl:gl + 1, :], rb[:])

            toggle = 0
            pending = None
            for ke in range(nge):
                c0, c1 = geb[ke], geb[ke + 1]
                w = (c1 - c0) * H
                l_ps = ps_l.tile([P, GE * H], F32)
                for q0 in range(c0, c1, 4):
                    q1 = min(c1, q0 + 4)
                    qw = (q1 - q0) * P
                    vt_ps = ps_vt.tile([P, 4 * P], F32)
                    for j in range(q0, q1):
                        nc.tensor.transpose(
                            vt_ps[:, (j - q0) * P:(j - q0 + 1) * P], vsl(j), ident[:]
                        )
                    vt_sb = vt4p.tile([P, 4 * P], F32)
                    if toggle:
                        nc.vector.tensor_copy(vt_sb[:, :qw], vt_ps[:, :qw])
                    else:
                        nc.scalar.activation(
                            vt_sb[:, :qw], vt_ps[:, :qw],
                            mybir.ActivationFunctionType.Copy,
                        )
                    toggle ^= 1
                    for j in range(q0, q1):
                        off = (j - c0) * H
                        nc.tensor.matmul(
                            l_ps[:, off:off + H],
                            vt_sb[:, (j - q0) * P:(j - q0 + 1) * P], w2_ap,
                            start=True, stop=False,
                        )
                        nc.tensor.matmul(
                            l_ps[:, off:off + H], onesrow[:], b2_ap,
                            start=False, stop=True,
                        )
                e1 = e12p.tile([P, GE * H], F32)
                e2 = e12p.tile([P, GE * H], F32)
                m12 = e12p.tile([P, GE * H], F32)
                nc.scalar.activation(
                    e1[:, :w], l_ps[:, :w], mybir.ActivationFunctionType.Exp
                )
                nc.scalar.activation(
                    e2[:, :w], l_ps[:, :w], mybir.ActivationFunctionType.Exp,
                    scale=ALPHA,
                )
                nc.vector.tensor_max(m12[:, :w], e1[:, :w], e2[:, :w])
                nc.vector.tensor_mul(
                    eall_sb[:, c0 * H:c0 * H + w], m12[:, :w],
                    em_sb[:, c0 * H:c0 * H + w],
                )
                # one-group-late accum emission keeps the in-order PE queue
                # from stalling on the exp chain mid-stream; in the last few
                # (small) groups PE is idle anyway, so emit immediately to
                # avoid queueing tail work behind later transposes
                if ke >= nge - 3:
                    if pending is not None:
                        emit_ready(*pending)
                        pending = None
                    emit_ready(c0, c1)
                else:
                    if pending is not None:
                        emit_ready(*pending)
                    pending = (c0, c1)
            if pending is not None:
                emit_ready(*pending)

    nc.compile()
    return nc


def _host_inputs(V, graph_size, weight, bias, tune_weight, order, caps, offs, nchunk):
    tw = np.asarray(tune_weight, np.float32)[0]                      # [H, C]
    wr = np.asarray(weight, np.float32).reshape(C, H, C)
    w2 = np.einsum("chd,hd->ch", wr, tw).astype(np.float32)          # [C, H]
    b2 = np.einsum("hd,hd->h", np.asarray(bias, np.float32).reshape(H, C), tw)
    b2r = b2.astype(np.float32).reshape(1, H)

    wb = np.zeros((C, 2 * H), np.float32)
    wb[:, :H] = w2
    wb[0, H:2 * H] = b2r[0]

    gs = np.asarray(graph_size, np.int64)
    in_maps = []
    core_graphs = []
    for c in range(NCORES):
        graphs = [int(order[g * NCORES + c]) for g in range(NSLOTS)]
        core_graphs.append(graphs)
        vcat = np.concatenate(
            [V[b, : caps[g] * P, :] for g, b in enumerate(graphs)], axis=0
        ).astype(np.float32, copy=False)
        vh = np.ascontiguousarray(
            vcat.reshape(nchunk, P, C).transpose(1, 0, 2).reshape(P, nchunk * C)
        )
        mask = np.zeros((P, nchunk), np.float32)
        prow = np.arange(P)
        for g, b in enumerate(graphs):
            for j in range(caps[g]):
                mask[(j * P + prow) < gs[b], offs[g] + j] = 1.0
        em = np.repeat(mask, H, axis=1)
        in_maps.append(
            {
                "Vh": vh,
                "em": em,
                "wb": wb,
                "wmat": np.ascontiguousarray(np.asarray(weight, np.float32)),
            }
        )
    return in_maps, core_graphs


def _assemble(results, core_graphs, caps, offs, nchunk, bias):
    bias = np.asarray(bias, np.float32)
    out = np.empty((B, H * C), np.float32)
    for c in range(NCORES):
        raw = np.asarray(results[c]["raw"])                    # [NSLOTS*H, H*C]
        zc = np.asarray(results[c]["zc"])                      # [P, NSLOTS]
        for g, b in enumerate(core_graphs[c]):
            Z = zc[: caps[g] * H, g].reshape(caps[g], H).sum(axis=0)   # [H]
            for h in range(H):
                blk = slice(h * C, (h + 1) * C)
                # last slot packs all head blocks into its first raw row
                r = g * H if g == NSLOTS - 1 else g * H + h
                out[b, blk] = raw[r, blk] / Z[h] + bias[blk]
    return out


def kernel(V, graph_size, weight, bias, tune_weight, _run=None):
    order, caps, offs, nchunk = _plan(graph_size)
    nc = _build_program(caps, nchunk)
    in_maps, core_graphs = _host_inputs(
        V, graph_size, weight, bias, tune_weight, order, caps, offs, nchunk
    )
    if _run is None:
        _run = lambda nc, in_maps: run_bass_kernel_spmd(
            nc, in_maps, list(range(NCORES))
        ).results
    results = _run(nc, in_maps)
    return _assemble(results, core_graphs, caps, offs, nchunk, bias)

